# revision 1
# baseline (speedup 1.0000x reference)
"""Per-sample modulated conv2d (StyleGAN2-style Conv2dMod) on 8 trn2 NeuronCores.

Reference computation (fp32):
    scale[n,o] = (1+y[n,o]) * rsqrt(||W[o]||^2 * (1+y[n,o])^2 + 1e-8)
    out = conv2d(edge_pad(x), W) * scale[:, :, None, None]

Sharding: data-parallel over the batch dim N=8 -> core n computes sample n's
full conv (C_in=512 -> C_out=512, 3x3, 32x32 spatial).

Per-core kernel: conv as PSUM-accumulated matmuls.
  - contraction dim = input channels (4 chunks of 128 partitions) x 9 taps
  - lhsT (stationary) = W[kh,kw][i_chunk 128, o_chunk 128]
  - rhs  (moving)     = padded x[i_chunk 128, h0+kh : h0+kh+16, kw : kw+32]
    (the 3x3 shifts are plain strided APs into one SBUF-resident [34,34] image)
  - psum [o_chunk 128, 2 x 512 pix] (2 banks) accumulates all 72 matmuls of an
    o_chunk -> scaled copy to SBUF (vector engine) -> DMA out.
288 matmuls of [128x128] @ [128x512] per core total.

Inputs are DMA'd in 16 weight chunks (o_chunk x i_chunk) + 4 x chunks
(i_chunk), emitted in exactly the order the matmul stream consumes them, so
the PE starts after ~1.2MB instead of ~4.7MB and the rest of the input
streams in behind the compute.
"""

import os

import numpy as np

N, C_IN, H, W = 8, 512, 32, 32
C_OUT, K = 512, 3
KK = K * K
EPS = 1e-08
HP, WP = H + 2, W + 2  # 34x34 edge-padded
IC, OC = C_IN // 128, C_OUT // 128  # 4 input / 4 output channel chunks
HB = 2  # pixel blocks: h in [0,16) and [16,32), 512 pixels each
RPB = H // HB  # rows per block

# matmul dtypes: "float32" (exact, 4 cyc/row), "float32r" (fp32 bits, tf32-like
# precision, 1 cyc/row), "bfloat16" (half DMA, ~2e-3 rel err).
# X_DTYPE = moving operand (activations), W_DTYPE = stationary (weights).
MM_DTYPE = os.environ.get("CONV_MM_DTYPE", "float32r")
X_DTYPE = os.environ.get("CONV_X_DTYPE", MM_DTYPE)
W_DTYPE = os.environ.get("CONV_W_DTYPE", MM_DTYPE)


def _build_bass():
    import concourse.bass as bass  # noqa: F401
    import concourse.mybir as mybir
    import concourse.tile as tile
    from concourse import bacc

    f32 = mybir.dt.float32
    xdt = getattr(mybir.dt, X_DTYPE)
    wdt = getattr(mybir.dt, W_DTYPE)

    # Bacc (not plain Bass): its finalize() runs move_matmul_waits_to_ldweights
    # + generate_event_semaphores, without which a matmul carrying >1 sem wait
    # fails walrus codegen ("Too many sync wait commands").
    nc = bacc.Bacc("TRN2")

    # [partition p = i%128, i_chunk, h, w] padded input for this core's sample
    xp_d = nc.dram_tensor("xp", [128, IC, HP, WP], xdt, kind="ExternalInput")
    # [partition p = i%128, o_chunk, i_chunk, tap kh*3+kw, o%128] weights
    wt_d = nc.dram_tensor("wt", [128, OC, IC, KK, 128], wdt, kind="ExternalInput")
    # [partition p = o%128, o_chunk] demod scale for this core's sample
    sc_d = nc.dram_tensor("sc", [128, OC], f32, kind="ExternalInput")
    # [o_chunk, o%128, pix] conv output * scale
    out_d = nc.dram_tensor("out", [OC, 128, H * W], f32, kind="ExternalOutput")

    WARM_MMS = int(os.environ.get("CONV_WARM_MMS", "56"))

    with tile.TileContext(nc) as tc:
        with (
            tc.tile_pool(name="singles", bufs=1) as singles,
            tc.tile_pool(name="psum", bufs=3, space="PSUM") as psum,
            tc.tile_pool(name="warmp", bufs=1, space="PSUM") as warmp,
            tc.tile_pool(name="outs", bufs=2) as outs,
        ):
            sc_s = singles.tile([128, OC], f32)
            nc.gpsimd.dma_start(out=sc_s, in_=sc_d[:])

            # PE warm-up: a stream of tiny self-contained matmuls on zeroed
            # SBUF fills the HAM activity window during the initial DMA wait,
            # so the real matmul stream starts at 2.4GHz instead of 1.2GHz
            # (saves ~5us of cold-clock penalty).
            if WARM_MMS:
                wdum = singles.tile([128, 128], f32, name="wdum")
                nc.vector.memset(wdum, 0.0)
                wdum_r = wdum.bitcast(xdt) if X_DTYPE == "float32r" else wdum
                warm_ps = warmp.tile([128, 128], f32, name="warm_ps")
                for _ in range(WARM_MMS):
                    nc.tensor.matmul(
                        warm_ps[:32, :],
                        wdum_r[:, :32],
                        wdum_r[:, :],
                        start=True,
                        stop=True,
                    )

            # Input-chunk tiles, DMA'd in exactly matmul-consumption order.
            # Concurrent DMA queues fair-share HBM bandwidth, so an
            # unconstrained launch burst starves the chunks the PE needs
            # first.  Chain each launch on the completion of the launch
            # CONC slots earlier: at most CONC transfers are ever in
            # flight, so early chunks get ~1/CONC of peak each and arrive
            # in order.  Launches alternate between the two DGE-capable
            # engines so launch issue isn't itself a serial bottleneck.
            from concourse.tile_rust import add_dep_helper

            CONC = int(os.environ.get("CONV_DMA_CONC", "4"))
            dma_chain = []

            def chain_dma(eng_idx, out, in_):
                eng = (nc.sync, nc.scalar)[eng_idx % 2]
                bi = eng.dma_start(out=out, in_=in_)
                i = len(dma_chain)
                if i >= CONC:
                    add_dep_helper(
                        bi.ins,
                        dma_chain[i - CONC].ins,
                        sync=True,
                        reason="dma pacing",
                    )
                dma_chain.append(bi)

            xp_t = [None] * IC
            wt_t = [[None] * IC for _ in range(OC)]

            def load_x(ic, split=False):
                t = singles.tile([128, HP, WP], xdt, tag=f"xp{ic}", name=f"xp{ic}")
                if split:
                    chain_dma(0, t[:, : HP // 2], xp_d[:, ic, : HP // 2])
                    chain_dma(1, t[:, HP // 2 :], xp_d[:, ic, HP // 2 :])
                else:
                    chain_dma(len(dma_chain), t, xp_d[:, ic])
                xp_t[ic] = t

            def load_w(oc, ic, ngroups=1):
                # ngroups>1 splits the chunk into tap-groups so the very
                # first matmuls only wait on a ~200KB transfer
                t = singles.tile(
                    [128, KK, 128], wdt, tag=f"wt{oc}_{ic}", name=f"wt{oc}_{ic}"
                )
                for g in range(ngroups):
                    k0, k1 = g * KK // ngroups, (g + 1) * KK // ngroups
                    chain_dma(len(dma_chain), t[:, k0:k1, :], wt_d[:, oc, ic, k0:k1])
                wt_t[oc][ic] = t

            load_x(0, split=True)
            load_w(0, 0, ngroups=3)
            for ic in range(1, IC):
                load_x(ic)
                load_w(0, ic, ngroups=2)
            for ic in range(IC):
                load_w(1, ic, ngroups=2)
            for oc in range(2, OC):
                for ic in range(IC):
                    load_w(oc, ic)

            for oc in range(OC):
                ps = psum.tile([128, HB, 512], f32, tag="ps", name="ps")
                # last o_chunk: finish pixel-block 0 first so its eviction +
                # store overlap block 1's matmuls (shorter kernel tail);
                # other o_chunks interleave blocks to halve the weight-DMA
                # demand rate at startup
                hb_phases = [(0, 1)] if oc < OC - 1 else [(0,), (1,)]
                for phase in hb_phases:
                    for ic in range(IC):
                        for khw in range(KK):
                            kh, kw = divmod(khw, K)
                            for hb in phase:
                                r0 = hb * RPB + kh
                                nc.tensor.matmul(
                                    ps[:, hb, :],
                                    wt_t[oc][ic][:, khw, :],
                                    xp_t[ic][:, r0 : r0 + RPB, kw : kw + W],
                                    start=(ic == 0 and khw == 0),
                                    stop=(ic == IC - 1 and khw == KK - 1),
                                )
                # out = psum * scale[n, oc*128+p], evicted by the vector engine
                o_t = outs.tile([128, HB * 512], f32, tag="o_t", name="o_t")
                if oc < OC - 1:
                    nc.vector.tensor_scalar_mul(o_t, ps, sc_s[:, oc : oc + 1])
                    nc.sync.dma_start(out=out_d[oc], in_=o_t)
                else:
                    for hb in range(HB):
                        nc.vector.tensor_scalar_mul(
                            o_t[:, hb * 512 : (hb + 1) * 512],
                            ps[:, hb, :],
                            sc_s[:, oc : oc + 1],
                        )
                        nc.sync.dma_start(
                            out=out_d[oc, :, hb * 512 : (hb + 1) * 512],
                            in_=o_t[:, hb * 512 : (hb + 1) * 512],
                        )

    nc.finalize()
    return nc


def _prep_host(x: np.ndarray, y: np.ndarray, weight: np.ndarray):
    """Shard + lay out inputs for the 8 cores. All fp32 numpy."""
    # demod scale, matching the fp32 reference math
    s = y + 1.0  # [N, O]
    wsq = np.sum(weight * weight, axis=(1, 2, 3))  # [O]
    scale = s / np.sqrt(wsq[None, :] * (s * s) + EPS)  # [N, O]

    # edge-replicate pad -> [N, C, 34, 34], then [p, ic, h, w] per core
    xp = np.pad(x, ((0, 0), (0, 0), (1, 1), (1, 1)), mode="edge")
    xp = xp.reshape(N, IC, 128, HP, WP).transpose(0, 2, 1, 3, 4)  # [N, p, ic, h, w]

    # weight[o, i, kh, kw] -> wt[p=i%128, oc, ic, khw, o%128]
    wt = weight.reshape(OC, 128, IC, 128, KK)  # [oc, o_lo, ic, p, khw]
    wt = wt.transpose(3, 0, 2, 4, 1)  # [p, oc, ic, khw, o_lo]
    wt = np.ascontiguousarray(wt)

    sc = scale.reshape(N, OC, 128).transpose(0, 2, 1)  # [N, p, oc]
    return xp, wt, sc


def kernel(x: np.ndarray, y: np.ndarray, weight: np.ndarray) -> np.ndarray:
    from concourse.bass_utils import run_bass_kernel_spmd

    x = np.asarray(x, dtype=np.float32)
    y = np.asarray(y, dtype=np.float32)
    weight = np.asarray(weight, dtype=np.float32)

    xp, wt, sc = _prep_host(x, y, weight)

    if X_DTYPE == "bfloat16":
        import ml_dtypes

        xp = xp.astype(ml_dtypes.bfloat16)
    if W_DTYPE == "bfloat16":
        import ml_dtypes

        wt = wt.astype(ml_dtypes.bfloat16)

    nc = _build_bass()
    in_maps = [
        {"xp": np.ascontiguousarray(xp[n]), "wt": wt, "sc": np.ascontiguousarray(sc[n])}
        for n in range(N)
    ]
    results = run_bass_kernel_spmd(nc, in_maps, core_ids=list(range(N))).results

    out = np.stack([r["out"].reshape(C_OUT, H, W) for r in results])
    return out.astype(np.float32)

